# revision 19
# baseline (speedup 1.0000x reference)
"""Trainium2 Bass kernel: multi-head causal attention (B=2, T=2048, C=1024, H=16).

Sharding: 8 cores = data parallel over B (2) x tensor parallel over head
groups (4 groups of 4 heads).  Each core computes its batch's partial
output contribution from its 4 heads through Wo rows; the host sums the 4
partials per batch (the "all-reduce") and adds the folded biases.

Device pipeline (per core, 4 heads, all matmul operands bf16 / PSUM fp32):
  - Q/K/V arrive bf16 [T, C]; DMA xbar-transpose loads them as [C, T] chunks
  - qT/kT = W^T @ X^T + b  laid out [head_dim, T];  v kept natural [T, dv]
    (bias bv folded on host: attn rows sum to 1 so attn@(v+bv) = attn@v + bv,
    and bv@Wo + bo is added during the host combine)
  - scores[q,k] accumulate in a [128, T] PSUM strip; additive -1e9 upper-tri
    mask on the diagonal 128-block (DVE); one Exp over the whole causal row
    with accumulated row-sum (ACT); reciprocal + row scale (DVE)
  - 128x128 PE transposes -> attnT, copied to SBUF in merged groups of 4
  - outT[dv, q] = sum_k v.T @ attnT accumulated in PSUM, heads paired for the
    output projection so its contraction runs at K=128
"""

from contextlib import ExitStack

import numpy as np
import ml_dtypes

import concourse.bass as bass
import concourse.mybir as mybir
import concourse.tile as tile
from concourse import bacc
from concourse.bass_utils import run_bass_kernel_spmd

B, T, C = 2, 2048, 1024
H, DK, DV = 16, 64, 64
N_CORES = 8
GROUPS = 4                 # head groups (tensor parallel)
HPG = H // GROUPS          # 4 heads per group
GD = HPG * DK              # 256 head dims per group
P = 128
TCH = 512                  # chunk of T for wide matmuls

BF = mybir.dt.bfloat16
F32 = mybir.dt.float32
AX = mybir.AxisListType
AF = mybir.ActivationFunctionType

bf16 = ml_dtypes.bfloat16


def _emit(nc, tc, io, t_len, ctx):
    NT = t_len // P            # query/key 128-blocks
    NTC = t_len // TCH         # 512-chunks
    NCB = C // P               # contraction chunks over C

    cpool = ctx.enter_context(tc.tile_pool(name="const", bufs=1))
    spool = ctx.enter_context(tc.tile_pool(name="stream", bufs=2))
    ppool = ctx.enter_context(tc.tile_pool(name="pers", bufs=1))
    apool = ctx.enter_context(tc.tile_pool(name="attn", bufs=2))
    pp = ctx.enter_context(tc.tile_pool(name="ps", bufs=2, space="PSUM"))

    # ---- constants / weights ------------------------------------------------
    ident = cpool.tile([P, P], BF)
    nc.sync.dma_start(out=ident, in_=io["ident"][:, :])
    amask = cpool.tile([P, P], F32)   # strict upper triangular -1e9
    nc.sync.dma_start(out=amask, in_=io["amask"][:, :])
    bq_sb = cpool.tile([P, 2], F32)
    nc.sync.dma_start(out=bq_sb, in_=io["bq"][:, :])
    bk_sb = cpool.tile([P, 2], F32)
    nc.sync.dma_start(out=bk_sb, in_=io["bk"][:, :])

    wq_sb = cpool.tile([P, NCB, GD], BF)
    wk_sb = cpool.tile([P, NCB, GD], BF)
    wv_sb = cpool.tile([P, NCB, GD], BF)
    for w_sb, name in ((wq_sb, "wq"), (wk_sb, "wk"), (wv_sb, "wv")):
        for cb in range(NCB):
            nc.sync.dma_start(out=w_sb[:, cb, :], in_=io[name][cb * P:(cb + 1) * P, :])
    wo_sb = cpool.tile([P, 2, C], BF)
    for pr in range(2):
        nc.sync.dma_start(out=wo_sb[:, pr, :], in_=io["wo"][pr * P:(pr + 1) * P, :])

    # persistent activations
    qT_sb = ppool.tile([P, 2, t_len], BF)   # [pair head dims(128), pair, T]
    kT_sb = ppool.tile([P, 2, t_len], BF)
    v_sb = ppool.tile([P, NT, GD], BF)      # natural [T(k), head dims]
    outT_sb = ppool.tile([P, 2, t_len], BF)  # [2 heads' dv, pair, T]

    # ---- stage 1+2: transposed loads + projections for one t-chunk ----------
    def load_t4(t4):
        for name, w_sb, bias_sb, xT_sb in (
            ("q", wq_sb, bq_sb, qT_sb),
            ("k", wk_sb, bk_sb, kT_sb),
            ("v", wv_sb, None, None),
        ):
            rows = spool.tile([P, 4, C], BF, tag="rows")
            for tb in range(4):
                r0 = (t4 * 4 + tb) * P
                nc.sync.dma_start(out=rows[:, tb, :], in_=io[name][r0:r0 + P, :])
            tch = spool.tile([P, NCB, TCH], BF, tag="tch")
            for cb in range(NCB):
                trp = pp.tile([P, TCH], BF, tag="tr", bufs=2)
                for tb in range(4):
                    nc.tensor.transpose(
                        trp[:, tb * P:(tb + 1) * P],
                        rows[:, tb, cb * P:(cb + 1) * P], ident)
                if cb % 2 == 0:
                    nc.vector.tensor_copy(tch[:, cb, :], trp)
                else:
                    nc.scalar.copy(tch[:, cb, :], trp)
            if name == "v":
                for tb in range(4):
                    ps = pp.tile([P, GD], F32, tag="mm", bufs=2)
                    for cb in range(NCB):
                        nc.tensor.matmul(
                            ps, tch[:, cb, tb * P:(tb + 1) * P], w_sb[:, cb, :],
                            start=(cb == 0), stop=(cb == NCB - 1))
                    nc.vector.tensor_copy(v_sb[:, t4 * 4 + tb, :], ps)
            else:
                for pr in range(2):
                    ps = pp.tile([P, TCH], F32, tag="mm", bufs=2)
                    for cb in range(NCB):
                        nc.tensor.matmul(
                            ps, w_sb[:, cb, pr * P:(pr + 1) * P], tch[:, cb, :],
                            start=(cb == 0), stop=(cb == NCB - 1))
                    nc.vector.tensor_scalar_add(
                        xT_sb[:, pr, t4 * TCH:(t4 + 1) * TCH], ps,
                        bias_sb[:, pr:pr + 1])

    # ---- stage 3+4: attention per (query chunk, head pair), then project ----
    SCW = 1024                 # width of one PSUM score strip (2 banks)

    def attend_qc(qc):
        nkb = (qc + 1) * 4
        for pr in range(2):
            attnTs = []
            for half in range(2):
                attnT = apool.tile([P, nkb, TCH], BF, tag="attnT", bufs=3)
                attnTs.append(attnT)
                # zero the never-written above-diagonal corners of the last 3
                # key blocks so full-width attn@v matmuls read zeros there
                for kd in range(1, 4):
                    kb = qc * 4 + kd
                    if kb < nkb:
                        nc.gpsimd.memset(attnT[:, kb, 0:kd * P], 0.0)
            for qs in range(4):
                qi = qc * 4 + qs
                kw = (qi + 1) * P
                nsc = (kw + SCW - 1) // SCW
                for half in range(2):
                    hs = half * DK
                    attnT = attnTs[half]
                    attn = apool.tile([P, t_len], BF, tag="attn", bufs=4)
                    sums = apool.tile([P, 2], F32, tag="S", bufs=6)
                    for si in range(nsc):
                        w = min(SCW, kw - si * SCW)
                        sc = pp.tile([P, SCW], F32, tag="sc", bufs=2)
                        for ci in range(0, w, TCH):
                            cw = min(TCH, w - ci)
                            nc.tensor.matmul(
                                sc[:, ci:ci + cw],
                                qT_sb[hs:hs + DK, pr, qi * P:(qi + 1) * P],
                                kT_sb[hs:hs + DK, pr, si * SCW + ci:si * SCW + ci + cw])
                        if si == nsc - 1:
                            # additive causal mask on the diagonal 128-block
                            nc.vector.tensor_add(sc[:, w - P:w], sc[:, w - P:w], amask)
                        nc.scalar.activation(
                            attn[:, si * SCW:si * SCW + w], sc[:, :w], AF.Exp,
                            scale=0.125, accum_out=sums[:, si:si + 1])
                    if nsc > 1:
                        S = apool.tile([P, 1], F32, tag="St", bufs=6)
                        nc.vector.reduce_sum(S, sums[:, :nsc], axis=AX.X)
                    else:
                        S = sums[:, 0:1]
                    R = apool.tile([P, 1], F32, tag="R", bufs=6)
                    nc.vector.reciprocal(R, S)
                    nc.vector.tensor_scalar_mul(attn[:, :kw], attn[:, :kw], R)
                    for g0 in range(0, qi + 1, 4):
                        n = min(4, qi + 1 - g0)
                        trp = pp.tile([P, TCH], BF, tag="tr", bufs=2)
                        for j in range(n):
                            kb = g0 + j
                            nc.tensor.transpose(
                                trp[:, j * P:(j + 1) * P],
                                attn[:, kb * P:(kb + 1) * P], ident)
                        nc.vector.tensor_copy(
                            attnT[:, g0:g0 + n, qs * P:(qs + 1) * P],
                            trp[:, :n * P].rearrange("p (a b) -> p a b", a=n))
            for half in range(2):
                av = pp.tile([DV, TCH], F32, tag="mm", bufs=2)
                for kb in range(nkb):
                    nc.tensor.matmul(
                        av, v_sb[:, kb, (pr * 2 + half) * DV:(pr * 2 + half + 1) * DV],
                        attnTs[half][:, kb, :],
                        start=(kb == 0), stop=(kb == nkb - 1))
                nc.vector.tensor_copy(
                    outT_sb[half * DV:(half + 1) * DV, pr, qc * TCH:(qc + 1) * TCH], av)
        # output projection for this query chunk (all 4 heads now done)
        for tb in range(qc * 4, qc * 4 + 4):
            fin = apool.tile([P, C], F32, tag="fin")
            for cc in range(C // TCH):
                ps = pp.tile([P, TCH], F32, tag="mm", bufs=2)
                for pr in range(2):
                    nc.tensor.matmul(
                        ps, outT_sb[:, pr, tb * P:(tb + 1) * P],
                        wo_sb[:, pr, cc * TCH:(cc + 1) * TCH],
                        start=(pr == 0), stop=(pr == 1))
                dst = fin[:, cc * TCH:(cc + 1) * TCH]
                if cc % 2 == 0:
                    nc.vector.tensor_copy(dst, ps)
                else:
                    nc.scalar.copy(dst, ps)
            nc.sync.dma_start(out=io["out"][tb * P:(tb + 1) * P, :], in_=fin)

    # emission order: PIPELINE interleaves loads and attention one chunk ahead
    PIPELINE = False
    if PIPELINE and NTC > 1:
        load_t4(0)
        load_t4(1)
        attend_qc(0)
        for t4 in range(2, NTC):
            load_t4(t4)
            attend_qc(t4 - 1)
        for qc in range(NTC - 1, NTC):
            attend_qc(qc)
    else:
        for t4 in range(NTC):
            load_t4(t4)
        for qc in range(NTC):
            attend_qc(qc)


def _build(t_len=T, reps=1):
    nc = bacc.Bacc("TRN2", target_bir_lowering=False, debug=False,
                   num_devices=N_CORES)
    io = {
        "q": nc.dram_tensor("q", [t_len, C], BF, kind="ExternalInput"),
        "k": nc.dram_tensor("k", [t_len, C], BF, kind="ExternalInput"),
        "v": nc.dram_tensor("v", [t_len, C], BF, kind="ExternalInput"),
        "wq": nc.dram_tensor("wq", [C, GD], BF, kind="ExternalInput"),
        "wk": nc.dram_tensor("wk", [C, GD], BF, kind="ExternalInput"),
        "wv": nc.dram_tensor("wv", [C, GD], BF, kind="ExternalInput"),
        "wo": nc.dram_tensor("wo", [GD, C], BF, kind="ExternalInput"),
        "bq": nc.dram_tensor("bq", [P, 2], F32, kind="ExternalInput"),
        "bk": nc.dram_tensor("bk", [P, 2], F32, kind="ExternalInput"),
        "ident": nc.dram_tensor("ident", [P, P], BF, kind="ExternalInput"),
        "amask": nc.dram_tensor("amask", [P, P], F32, kind="ExternalInput"),
        "out": nc.dram_tensor("out", [t_len, C], F32, kind="ExternalOutput"),
    }
    with tile.TileContext(nc) as tc, ExitStack() as ctx:
        if reps == 1:
            _emit(nc, tc, io, t_len, ctx)
        else:
            hints = (mybir.EngineType.PE, mybir.EngineType.DVE,
                     mybir.EngineType.Activation, mybir.EngineType.Pool,
                     mybir.EngineType.SP)
            with tc.For_i(0, reps, 1, hint_engines=hints):
                _emit(nc, tc, io, t_len, ctx)
    nc.compile()
    return nc


_NC_CACHE = {}


def _get_nc(t_len=T, reps=1):
    key = (t_len, reps)
    if key not in _NC_CACHE:
        _NC_CACHE[key] = _build(t_len, reps)
    return _NC_CACHE[key]


def _host_constants():
    ident = np.eye(P, dtype=bf16)
    amask = np.triu(np.full((P, P), -1e9, np.float32), 1)
    return ident, amask


def make_in_maps(inputs, t_len=T):
    Q, K, V = inputs["Q"], inputs["K"], inputs["V"]
    Wq, bq = inputs["Wq"], inputs["bq"]
    Wk, bk = inputs["Wk"], inputs["bk"]
    Wv = inputs["Wv"]
    Wo = inputs["Wo"]
    ident, amask = _host_constants()
    in_maps = []
    for core in range(N_CORES):
        b, g = divmod(core, GROUPS)
        cs = slice(g * GD, (g + 1) * GD)
        in_maps.append({
            "q": np.ascontiguousarray(Q[b, :t_len]).astype(bf16),
            "k": np.ascontiguousarray(K[b, :t_len]).astype(bf16),
            "v": np.ascontiguousarray(V[b, :t_len]).astype(bf16),
            "wq": np.ascontiguousarray(Wq[:, cs]).astype(bf16),
            "wk": np.ascontiguousarray(Wk[:, cs]).astype(bf16),
            "wv": np.ascontiguousarray(Wv[:, cs]).astype(bf16),
            "wo": np.ascontiguousarray(Wo[cs, :]).astype(bf16),
            "bq": np.ascontiguousarray(bq[cs].reshape(2, P).T).astype(np.float32),
            "bk": np.ascontiguousarray(bk[cs].reshape(2, P).T).astype(np.float32),
            "ident": ident,
            "amask": amask,
        })
    return in_maps


def combine(results, inputs, t_len=T):
    bo, bv, Wo = inputs["bo"], inputs["bv"], inputs["Wo"]
    bias = (bo.astype(np.float64) + bv.astype(np.float64) @ Wo.astype(np.float64))
    out = np.empty((B, t_len, C), np.float32)
    for b in range(B):
        acc = np.zeros((t_len, C), np.float64)
        for g in range(GROUPS):
            acc += results[b * GROUPS + g]["out"].astype(np.float64)
        out[b] = (acc + bias).astype(np.float32)
    return out


def _mask_is_causal(mask, t_len):
    mask = np.asarray(mask)
    if mask.shape != (1, 1, t_len, t_len):
        return False
    m = mask[0, 0]
    tri = np.tril(np.ones((t_len, t_len), bool))
    return (m[tri] == 0.0).all() and (m[~tri] <= -1e8).all()


def _reference_fallback(inputs):
    # generic-mask fallback (never hit with the causal reference mask)
    Q, K, V = (np.asarray(inputs[k], np.float32) for k in ("Q", "K", "V"))
    mask = np.asarray(inputs["mask"], np.float32)
    out = np.empty((B, T, C), np.float32)
    for b in range(B):
        acc = np.zeros((T, C), np.float32)
        for h in range(H):
            q = Q[b] @ inputs["Wq"][:, h * DK:(h + 1) * DK] + inputs["bq"][h * DK:(h + 1) * DK]
            k = K[b] @ inputs["Wk"][:, h * DK:(h + 1) * DK] + inputs["bk"][h * DK:(h + 1) * DK]
            v = V[b] @ inputs["Wv"][:, h * DV:(h + 1) * DV] + inputs["bv"][h * DV:(h + 1) * DV]
            m = mask[min(b, mask.shape[0] - 1), min(h, mask.shape[1] - 1)]
            s = (q @ k.T + m) / np.sqrt(DK).astype(np.float32)
            s -= s.max(-1, keepdims=True)
            e = np.exp(s)
            a = e / e.sum(-1, keepdims=True)
            acc += (a @ v) @ inputs["Wo"][h * DV:(h + 1) * DV, :]
        out[b] = acc + inputs["bo"]
    return out


def kernel(**inputs):
    inputs = {k: np.asarray(v) for k, v in inputs.items()}
    if not _mask_is_causal(inputs["mask"], T):
        return _reference_fallback(inputs)
    nc = _get_nc(T)
    in_maps = make_in_maps(inputs, T)
    res = run_bass_kernel_spmd(nc, in_maps, core_ids=list(range(N_CORES)))
    return combine(res.results, inputs, T)
